# revision 6
# baseline (speedup 1.0000x reference)
"""Trainium2 Bass kernel for nn_BinarizeLayer.

out[b, f] = (medians[f] > 0) AND (inputs[b, f] >= medians[f])

Host preprocessing folds the two conditions into one comparison:
m2[f] = medians[f] if medians[f] > 0 else +inf, so out = inputs >= m2
(inputs are finite, so x >= +inf is always False).

Data-parallel over 8 NeuronCores: each core handles a 2048-row slice of
the 16384x8192 f32 input. Per 128-row tile the DVE compares against the
median row (replicated across partitions once, via on-chip doubling
DMAs) and bit-packs the 0/1 bytes 8-to-1, so each core stores 2 MiB
instead of 16 MiB. The host unpacks bits to the final bool array.
"""

import numpy as np

import concourse.bacc as bacc
import concourse.mybir as mybir
from concourse import tile
from concourse.bass_utils import run_bass_kernel_spmd

N_CORES = 8
B, F = 16384, 8192
BS = B // N_CORES  # rows per core
P = 128  # SBUF partitions
N_TILES = BS // P  # row-tiles per core
FP = F // 8  # packed bytes per row


def _build():
    nc = bacc.Bacc(
        "TRN2",
        target_bir_lowering=False,
        debug=False,
        num_devices=N_CORES,
    )
    x = nc.declare_dram_parameter("x", [BS, F], mybir.dt.float32, isOutput=False)
    med = nc.declare_dram_parameter("med", [1, F], mybir.dt.float32, isOutput=False)
    out = nc.declare_dram_parameter("out", [BS, FP], mybir.dt.uint8, isOutput=True)

    with tile.TileContext(nc) as tc:
        with (
            tc.tile_pool(name="const", bufs=1) as cpool,
            tc.tile_pool(name="io", bufs=4) as pool,
        ):
            # Replicate the median row to all 128 partitions by doubling
            # SBUF->SBUF DMAs (no HBM re-read).
            med_t = cpool.tile([P, F], mybir.dt.float32)
            nc.sync.dma_start(out=med_t[0:1, :], in_=med[:])
            k = 1
            while k < P:
                nc.sync.dma_start(out=med_t[k : 2 * k, :], in_=med_t[0:k, :])
                k *= 2

            for i in range(N_TILES):
                xt = pool.tile([P, F], mybir.dt.float32, tag="x")
                nc.sync.dma_start(out=xt[:], in_=x[i * P : (i + 1) * P, :])
                u8 = xt.bitcast(mybir.dt.uint8)  # [P, 4F] view
                # 0/1 bytes over the head of the tile (write trails read).
                nc.vector.tensor_tensor(
                    u8[:, :F], xt[:], med_t[:], mybir.AluOpType.is_ge
                )
                # Bit-pack 8:1 into disjoint scratch regions of the tile:
                # p1[j] = b[2j] + 2*b[2j+1]      -> [F : F+F/2)
                # p2[j] = p1[2j] + 4*p1[2j+1]    -> [F+F/2 : F+3F/4)
                # p3[j] = p2[2j] + 16*p2[2j+1]   -> [F+3F/4 : F+7F/8)
                r1, r2, r3 = F, F + F // 2, F + 3 * F // 4
                nc.vector.scalar_tensor_tensor(
                    u8[:, r1 : r1 + F // 2],
                    u8[:, 1:F:2],
                    2.0,
                    u8[:, 0:F:2],
                    mybir.AluOpType.mult,
                    mybir.AluOpType.add,
                )
                nc.vector.scalar_tensor_tensor(
                    u8[:, r2 : r2 + F // 4],
                    u8[:, r1 + 1 : r1 + F // 2 : 2],
                    4.0,
                    u8[:, r1 : r1 + F // 2 : 2],
                    mybir.AluOpType.mult,
                    mybir.AluOpType.add,
                )
                nc.vector.scalar_tensor_tensor(
                    u8[:, r3 : r3 + FP],
                    u8[:, r2 + 1 : r2 + F // 4 : 2],
                    16.0,
                    u8[:, r2 : r2 + F // 4 : 2],
                    mybir.AluOpType.mult,
                    mybir.AluOpType.add,
                )
                nc.sync.dma_start(
                    out=out[i * P : (i + 1) * P, :], in_=u8[:, r3 : r3 + FP]
                )
    nc.compile()
    return nc


def kernel(inputs, medians):
    x = np.ascontiguousarray(np.asarray(inputs, dtype=np.float32))
    m = np.asarray(medians, dtype=np.float32)
    m2 = np.where(m > 0, m, np.float32(np.inf)).astype(np.float32).reshape(1, F)

    nc = _build()
    in_maps = [{"x": x[c * BS : (c + 1) * BS], "med": m2} for c in range(N_CORES)]
    res = run_bass_kernel_spmd(nc, in_maps, list(range(N_CORES))).results
    packed = np.concatenate([r["out"] for r in res], axis=0)
    return np.unpackbits(packed, axis=1, bitorder="little").astype(bool)


# revision 10
# speedup vs baseline: 1.2170x; 1.2170x over previous
"""Trainium2 Bass kernel for nn_BinarizeLayer.

out[b, f] = (medians[f] > 0) AND (inputs[b, f] >= medians[f])

Host preprocessing folds the two conditions into one comparison:
m2[f] = medians[f] if medians[f] > 0 else +inf, so out = inputs >= m2
(inputs are finite, so x >= +inf is always False).

Data-parallel over 8 NeuronCores: each core handles a 2048-row slice of
the 16384x8192 f32 input. Per 128-row tile:
  - DVE compares against the median row (replicated across partitions
    once via on-chip doubling DMAs), emitting 0/1 bf16 bits;
  - the tensor engine bit-packs 8 batch rows per byte with one constant
    [128,16] matmul weight (2^(p%8) block-diagonal), accumulating exact
    small integers in PSUM;
  - the scalar engine evacuates PSUM to SBUF with an f32->u8 cast.
Each core stores 2 MiB of packed bytes instead of 16 MiB; the host
unpacks bits (along the batch axis) back to the full bool array.

Engine budget per core: DVE ~137us, PE ~60-110us, ACT ~125us, all under
the ~195-205us DMA time for 64 MiB in + 2 MiB out at ~358 GB/s per NC.
"""

import numpy as np

import concourse.bacc as bacc
import concourse.mybir as mybir
from concourse import tile
from concourse.bass_utils import run_bass_kernel_spmd

N_CORES = 8
B, F = 16384, 8192
BS = B // N_CORES  # rows per core
P = 128  # SBUF partitions
N_TILES = BS // P  # row-tiles per core
G = P // 8  # packed rows per tile (16)
MM_N = 512  # matmul free-dim chunk (one PSUM bank)
PS_W = 2048  # PSUM tile width (4 banks)


def _build():
    nc = bacc.Bacc(
        "TRN2",
        target_bir_lowering=False,
        debug=False,
        num_devices=N_CORES,
    )
    x = nc.declare_dram_parameter("x", [BS, F], mybir.dt.float32, isOutput=False)
    med = nc.declare_dram_parameter("med", [1, F], mybir.dt.float32, isOutput=False)
    pw = nc.declare_dram_parameter("pw", [P, G], mybir.dt.float32, isOutput=False)
    out = nc.declare_dram_parameter(
        "out", [BS // 8, F], mybir.dt.uint8, isOutput=True
    )

    with tile.TileContext(nc) as tc:
        with (
            tc.tile_pool(name="const", bufs=1) as cpool,
            tc.tile_pool(name="xp", bufs=3) as xpool,
            tc.tile_pool(name="bp", bufs=2) as bpool,
            tc.tile_pool(name="op", bufs=2) as opool,
            tc.tile_pool(name="ps", bufs=2, space="PSUM") as pspool,
        ):
            # Replicate the median row to all 128 partitions by doubling
            # SBUF->SBUF DMAs (no HBM re-read).
            med_t = cpool.tile([P, F], mybir.dt.float32)
            nc.sync.dma_start(out=med_t[0:1, :], in_=med[:])
            k = 1
            while k < P:
                nc.sync.dma_start(out=med_t[k : 2 * k, :], in_=med_t[0:k, :])
                k *= 2
            # Pack weights, cast to bf16 for the PE (values 2^k, exact).
            pw_f32 = cpool.tile([P, G], mybir.dt.float32)
            pw_t = cpool.tile([P, G], mybir.dt.bfloat16)
            nc.sync.dma_start(out=pw_f32[:], in_=pw[:])
            nc.vector.tensor_copy(out=pw_t[:], in_=pw_f32[:])

            for i in range(N_TILES):
                xt = xpool.tile([P, F], mybir.dt.float32, tag="x")
                nc.sync.dma_start(out=xt[:], in_=x[i * P : (i + 1) * P, :])
                bt = bpool.tile([P, F], mybir.dt.bfloat16, tag="b")
                nc.vector.tensor_tensor(
                    bt[:], xt[:], med_t[:], mybir.AluOpType.is_ge
                )
                pk = opool.tile([G, F], mybir.dt.uint8, tag="o")
                for c in range(0, F, PS_W):
                    ps = pspool.tile([G, PS_W], mybir.dt.float32, tag="ps")
                    for n in range(0, PS_W, MM_N):
                        nc.tensor.matmul(
                            ps[:, n : n + MM_N],
                            pw_t[:],
                            bt[:, c + n : c + n + MM_N],
                            start=True,
                            stop=True,
                        )
                    nc.scalar.copy(out=pk[:, c : c + PS_W], in_=ps[:])
                nc.sync.dma_start(out=out[i * G : (i + 1) * G, :], in_=pk[:])
    nc.compile()
    return nc


def _pack_weights():
    pw = np.zeros((P, G), dtype=np.float32)
    for p in range(P):
        pw[p, p // 8] = float(1 << (p % 8))
    return pw


def _in_maps(inputs, medians):
    x = np.ascontiguousarray(np.asarray(inputs, dtype=np.float32))
    m = np.asarray(medians, dtype=np.float32)
    m2 = np.where(m > 0, m, np.float32(np.inf)).astype(np.float32).reshape(1, F)
    pw = _pack_weights()
    return [
        {"x": x[c * BS : (c + 1) * BS], "med": m2, "pw": pw} for c in range(N_CORES)
    ]


def kernel(inputs, medians):
    nc = _build()
    res = run_bass_kernel_spmd(nc, _in_maps(inputs, medians), list(range(N_CORES))).results
    out = np.concatenate(
        [np.unpackbits(r["out"], axis=0, bitorder="little") for r in res], axis=0
    )
    return out.astype(bool)
